# revision 20
# baseline (speedup 1.0000x reference)
"""SAGAN-style attention block on 8 trn2 NeuronCores, batch-parallel.

Math per batch element (C=64, H=W=64, S=4096, T=S/4=1024):
  theta = w_theta @ x                      [8, S]
  phi   = maxpool2(w_phi @ x)              [8, T]
  g     = maxpool2(w_g @ x)                [32, T]
  beta  = softmax_t(theta^T @ phi)         [S, T]
  out   = gamma * (w_o @ (g @ beta^T)) + x [C, S]

The wall-clock of a call is dominated by the axon tunnel + per-call
jit/compile overhead, not device exec (~121us/core simulated), so the
kernel is shaped to minimize host I/O:
  - x is shipped fp8e4m3 (4.2MB instead of 16.8MB f32); the residual is
    added host-side from the exact f32 x, and fp8 noise only enters the
    attention branch, which is scaled by gamma=0.1 (measured end-to-end
    rel err 9.0e-3 vs the 2e-2 gate, matching a numpy simulation)
  - the device returns only the normalized pre-w_o attention tensor
    o2 = (g @ beta^T)/Z in fp8 [NB, 32, S] (2.1MB down + 2.1MB donated
    zero upload); the tiny w_o matmul, gamma scale and residual add run
    host-side (~40ms of numpy)
  - a dummy custom-DVE instruction routes per-call DVE table generation
    through the process-level dve_table_for_ops cache (saves ~0.3s/call
    of uncached default-table regen inside get_walrus_args); the real
    softmax normalization uses nc.vector.reciprocal, because
    reciprocal_approx_fast returns garbage on this terminal
  - identity block for the g-transpose rides in unused wcat columns, so
    there are only two input tensors

Device strategy (per core, 2 batch elements):
  - one fused conv matmul for theta/phi/g (96 padded channels: 0:8 theta,
    32:40 phi, 64:96 g so slices start at partition 0/32/64), bf16 in
  - scores computed TRANSPOSED [t, s] so the o-matmul needs no transpose
  - softmax without max-subtraction (scores are O(+-25), exp safe in f32)
  - Z = sum_t exp via a ones-column appended to the o-matmul lhsT
  - g transposed into the o-matmul lhsT via identity matmul
"""

import os
import sys

import numpy as np

os.environ.setdefault("JAX_PLATFORMS", "axon,cpu")
# smaller NEFF to repack+ship each call (debug info is never read here)
os.environ.setdefault("CONCOURSE_SCRUB_NEFF_DEBUG_INFO", "1")
for _p in ("/opt/trn_rl_repo",):
    if _p not in sys.path:
        sys.path.insert(0, _p)

import concourse.bacc as bacc
import concourse.tile as tile
from concourse import mybir
from concourse.bass_utils import run_bass_kernel_spmd

F32 = mybir.dt.float32
BF16 = mybir.dt.bfloat16
F8 = mybir.dt.float8e4
AX = mybir.AluOpType
EXP = mybir.ActivationFunctionType.Exp
BF16_NP = mybir.dt.np(mybir.dt.bfloat16)
F8_NP = mybir.dt.np(mybir.dt.float8e4)

NB = 2          # batch elements per core
C = 64
S = 4096        # H*W
T = 1024        # pooled spatial
SB = 512        # s-block width
NSB = S // SB   # 8
NTC = T // 128  # 8 t-chunks
GROUPS = [(0, 3), (3, 6), (6, 8)]  # t-chunk grouping for big ACT exp ops

_prog_cache = {}
last_results = None


def _build_program():
    nc = bacc.Bacc(None, target_bir_lowering=False, debug=False)
    xin = nc.dram_tensor("xin", [NB, C, S], F8, kind="ExternalInput")
    # cols 0:96 = fused conv weights; rows 0:32 of cols 96:128 = identity
    wcat = nc.dram_tensor("wcat", [C, 128], BF16, kind="ExternalInput")
    yout = nc.dram_tensor("yout", [NB, 32, S], F8, kind="ExternalOutput")

    with tile.TileContext(nc) as tc:
        with nc.allow_low_precision(reason="bf16 attention; residual is f32 host-side"):
            _body(tc, xin, wcat, yout)
    nc.compile()
    return nc


def _body(tc, xin, wcat, yout):
    nc = tc.nc
    with (
        tc.tile_pool(name="const", bufs=1) as cpool,
        tc.tile_pool(name="big", bufs=2) as bpool,
        tc.tile_pool(name="work", bufs=2) as wpool,
        tc.tile_pool(name="stexp", bufs=2) as epool,
        tc.psum_pool(name="ps_sc", bufs=2) as ps_sc,
        tc.psum_pool(name="ps_o", bufs=2) as ps_o,
    ):
        wcat_sb = cpool.tile([C, 128], BF16)
        nc.sync.dma_start(wcat_sb[:], wcat[:])
        ident_sb = wcat_sb[0:32, 96:128]
        ones_f = cpool.tile([128, 1], F32)
        nc.vector.memset(ones_f[:], 1.0)
        ones_sb = cpool.tile([1, 32], F32)
        nc.vector.tensor_copy(ones_sb[:], ones_f[0:1, :].to_broadcast([1, 32]))
        # dummy custom-DVE op (output unused): routes DVE table generation
        # through the process-cached dve_table_for_ops path (~0.3s/call saved)
        dve_dummy = cpool.tile([1, 1], F32)
        nc.vector.reciprocal_approx_fast(dve_dummy[:], ones_f[0:1, 0:1])

        for b in range(NB):
            x8_sb = bpool.tile([C, S], F8, tag="x8")
            nc.sync.dma_start(x8_sb[:], xin[b])
            x_sb = bpool.tile([C, S], BF16, tag="x")
            nc.vector.tensor_copy(x_sb[:], x8_sb[:])

            # fused 1x1 convs: rows 0:8 theta, 32:40 phi_pre, 64:96 g_pre
            pre_sb = bpool.tile([96, S], BF16, tag="pre")
            for j in range(NSB):
                cps = ps_sc.tile([96, SB], F32, tag="sc")
                nc.tensor.matmul(
                    cps[:], wcat_sb[:, 0:96], x_sb[:, j * SB:(j + 1) * SB],
                    start=True, stop=True,
                )
                nc.vector.tensor_copy(pre_sb[:, j * SB:(j + 1) * SB], cps[:])

            # 2x2 maxpool on phi_pre and g_pre (own tiles so base_partition=0)
            phi_sb = wpool.tile([8, T], BF16, tag="phi")
            g_sb = wpool.tile([32, T], BF16, tag="g")
            phm = wpool.tile([8, 2048], BF16, tag="phm")
            ghm = wpool.tile([32, 2048], BF16, tag="ghm")
            pv = pre_sb[32:40].rearrange("p (h w) -> p h w", h=64)
            nc.vector.tensor_tensor(
                phm[:].rearrange("p (h w) -> p h w", h=64),
                pv[:, :, 0:64:2], pv[:, :, 1:64:2], AX.max)
            ph2 = phm[:].rearrange("p (h w) -> p h w", h=64)
            nc.vector.tensor_tensor(
                phi_sb[:].rearrange("p (h w) -> p h w", h=32),
                ph2[:, 0:64:2, :], ph2[:, 1:64:2, :], AX.max)
            gv = pre_sb[64:96].rearrange("p (h w) -> p h w", h=64)
            nc.vector.tensor_tensor(
                ghm[:].rearrange("p (h w) -> p h w", h=64),
                gv[:, :, 0:64:2], gv[:, :, 1:64:2], AX.max)
            gh2 = ghm[:].rearrange("p (h w) -> p h w", h=64)
            nc.vector.tensor_tensor(
                g_sb[:].rearrange("p (h w) -> p h w", h=32),
                gh2[:, 0:64:2, :], gh2[:, 1:64:2, :], AX.max)

            # g2T chunks: [128 t, 33] = g[:, chunk].T via identity; col 32 = ones
            g2t_sb = bpool.tile([128, NTC * 33], BF16, tag="g2t")
            nc.vector.tensor_copy(
                g2t_sb[:].rearrange("p (k c) -> p k c", c=33)[:, :, 32],
                ones_f[:].to_broadcast([128, NTC]))
            for k in range(NTC):
                g2ps = ps_o.tile([128, 32], F32, tag="o")
                nc.tensor.matmul(
                    g2ps[:], g_sb[:, k * 128:(k + 1) * 128], ident_sb[:],
                    start=True, stop=True,
                )
                nc.vector.tensor_copy(g2t_sb[:, k * 33:k * 33 + 32], g2ps[:])

            theta = pre_sb[0:8]
            for j in range(NSB):
                st_exp = epool.tile([128, NTC * SB], BF16, tag="stexp")
                for (k0, k1) in GROUPS:
                    scps = ps_sc.tile([128, 3 * SB], F32, tag="sc")
                    for k in range(k0, k1):
                        nc.tensor.matmul(
                            scps[:, (k - k0) * SB:(k - k0 + 1) * SB],
                            phi_sb[:, k * 128:(k + 1) * 128],
                            theta[:, j * SB:(j + 1) * SB],
                            start=True, stop=True,
                        )
                    nc.scalar.activation(
                        st_exp[:, k0 * SB:k1 * SB],
                        scps[:, 0:(k1 - k0) * SB], EXP)

                o_ps = ps_o.tile([33, SB], F32, tag="o")
                for k in range(NTC):
                    nc.tensor.matmul(
                        o_ps[:],
                        g2t_sb[:, k * 33:(k + 1) * 33],
                        st_exp[:, k * SB:(k + 1) * SB],
                        start=(k == 0), stop=(k == NTC - 1),
                    )

                # free the o_ps slot with one fast copy; normalize off SBUF
                o_sb = wpool.tile([33, SB], F32, tag="osb")
                nc.vector.tensor_copy(o_sb[:], o_ps[:])
                zr = wpool.tile([1, SB], F32, tag="zr")
                nc.vector.reciprocal(zr[:], o_sb[32:33, :])
                # broadcast 1/Z across the 32 channel partitions via K=1 matmul
                zb_ps = ps_o.tile([32, SB], F32, tag="o")
                nc.tensor.matmul(
                    zb_ps[:], ones_sb[:], zr[:], start=True, stop=True)
                out_sb = wpool.tile([32, SB], F8, tag="out")
                nc.vector.tensor_tensor(out_sb[:], o_sb[0:32, :], zb_ps[:], AX.mult)
                nc.sync.dma_start(yout[b][:, j * SB:(j + 1) * SB], out_sb[:])


def kernel(x, w_theta, w_phi, w_g, w_o, gamma):
    global last_results
    x = np.ascontiguousarray(np.asarray(x, dtype=np.float32))
    B = x.shape[0]
    n_cores = 8
    per = B // n_cores
    assert per == NB

    if "prog" not in _prog_cache:
        _prog_cache["prog"] = _build_program()
    nc = _prog_cache["prog"]

    wcat_full = np.zeros((128, C), dtype=np.float32)
    wcat_full[0:8] = np.asarray(w_theta)
    wcat_full[32:40] = np.asarray(w_phi)
    wcat_full[64:96] = np.asarray(w_g)
    wcat_full[96:128, 0:32] = np.eye(32, dtype=np.float32)
    wcat_np = np.ascontiguousarray(wcat_full.T).astype(BF16_NP)

    x_f8 = x.reshape(B, C, S).astype(F8_NP)
    in_maps = [
        {
            "xin": x_f8[i * NB:(i + 1) * NB],
            "wcat": wcat_np,
        }
        for i in range(n_cores)
    ]
    res = run_bass_kernel_spmd(nc, in_maps, core_ids=list(range(n_cores)))
    last_results = res

    # o2 = (g @ beta^T)/Z from the device; w_o, gamma and the residual in f32
    o2f = np.empty((B, 32, S), dtype=np.float32)
    for i in range(n_cores):
        o2f[i * NB:(i + 1) * NB] = np.asarray(res.results[i]["yout"])
    w_og = (float(np.asarray(gamma)) * np.asarray(w_o)).astype(np.float32)
    out = np.matmul(w_og, o2f)
    out += x.reshape(B, C, S)
    return out.reshape(B, C, 64, 64)


# revision 36
# speedup vs baseline: 1.0200x; 1.0200x over previous
"""SAGAN-style attention block on 8 trn2 NeuronCores, batch-parallel.

Math per batch element (C=64, H=W=64, S=4096, T=S/4=1024):
  theta = w_theta @ x                      [8, S]
  phi   = maxpool2(w_phi @ x)              [8, T]
  g     = maxpool2(w_g @ x)                [32, T]
  beta  = softmax_t(theta^T @ phi)         [S, T]
  out   = gamma * (w_o @ (g @ beta^T)) + x [C, S]

The wall-clock of a call is dominated by the axon tunnel + per-call
jit/compile overhead, not device exec (~121us/core simulated), so the
kernel is shaped to minimize host I/O:
  - x is shipped fp8e4m3 (4.2MB instead of 16.8MB f32); the residual is
    added host-side from the exact f32 x, and fp8 noise only enters the
    attention branch, which is scaled by gamma=0.1 (measured end-to-end
    rel err 9.0e-3 vs the 2e-2 gate, matching a numpy simulation)
  - the device returns only the normalized pre-w_o attention tensor
    o2 = (g @ beta^T)/Z in fp8 [NB, 32, S] (2.1MB down + 2.1MB donated
    zero upload); the tiny w_o matmul, gamma scale and residual add run
    host-side (~40ms of numpy)
  - a dummy custom-DVE instruction routes per-call DVE table generation
    through the process-level dve_table_for_ops cache (saves ~0.3s/call
    of uncached default-table regen inside get_walrus_args); the real
    softmax normalization uses nc.vector.reciprocal, because
    reciprocal_approx_fast returns garbage on this terminal
  - identity block for the g-transpose rides in unused wcat columns, so
    there are only two input tensors

Device strategy (per core, 2 batch elements):
  - one fused conv matmul for theta/phi/g (96 padded channels: 0:8 theta,
    32:40 phi, 64:96 g so slices start at partition 0/32/64), bf16 in
  - scores computed TRANSPOSED [t, s] so the o-matmul needs no transpose
  - softmax without max-subtraction (scores are O(+-25), exp safe in f32)
  - Z = sum_t exp via a ones-column appended to the o-matmul lhsT
  - g transposed into the o-matmul lhsT via identity matmul
"""

import os
import sys

import numpy as np

os.environ.setdefault("JAX_PLATFORMS", "axon,cpu")
# smaller NEFF to repack+ship each call (debug info is never read here)
os.environ.setdefault("CONCOURSE_SCRUB_NEFF_DEBUG_INFO", "1")
for _p in ("/opt/trn_rl_repo",):
    if _p not in sys.path:
        sys.path.insert(0, _p)

import concourse.bacc as bacc
import concourse.tile as tile
from concourse import mybir
from concourse.bass_utils import run_bass_kernel_spmd

F32 = mybir.dt.float32
BF16 = mybir.dt.bfloat16
F8 = mybir.dt.float8e4
AX = mybir.AluOpType
EXP = mybir.ActivationFunctionType.Exp
BF16_NP = mybir.dt.np(mybir.dt.bfloat16)
F8_NP = mybir.dt.np(mybir.dt.float8e4)

NB = 2          # batch elements per core
C = 64
S = 4096        # H*W
T = 1024        # pooled spatial
SB = 512        # s-block width
NSB = S // SB   # 8
NTC = T // 128  # 8 t-chunks
GROUPS = [(0, 3), (3, 6), (6, 8)]  # t-chunk grouping for big ACT exp ops

_prog_cache = {}
last_results = None


def _build_program():
    nc = bacc.Bacc(None, target_bir_lowering=False, debug=False)
    xin = nc.dram_tensor("xin", [NB, C, S], F8, kind="ExternalInput")
    # cols 0:96 = fused conv weights; rows 0:32 of cols 96:128 = identity
    wcat = nc.dram_tensor("wcat", [C, 128], BF16, kind="ExternalInput")
    yout = nc.dram_tensor("yout", [NB, 32, S], F8, kind="ExternalOutput")

    with tile.TileContext(nc) as tc:
        with nc.allow_low_precision(reason="bf16 attention; residual is f32 host-side"):
            _body(tc, xin, wcat, yout)
    nc.compile()
    return nc


def _body(tc, xin, wcat, yout):
    nc = tc.nc
    with (
        tc.tile_pool(name="const", bufs=1) as cpool,
        tc.tile_pool(name="big", bufs=2) as bpool,
        tc.tile_pool(name="work", bufs=2) as wpool,
        tc.tile_pool(name="stexp", bufs=2) as epool,
        tc.psum_pool(name="ps_sc", bufs=2) as ps_sc,
        tc.psum_pool(name="ps_o", bufs=2) as ps_o,
    ):
        wcat_sb = cpool.tile([C, 128], BF16)
        nc.sync.dma_start(wcat_sb[:], wcat[:])
        ident_sb = wcat_sb[0:32, 96:128]
        ones_f = cpool.tile([128, 1], F32)
        nc.vector.memset(ones_f[:], 1.0)
        ones_sb = cpool.tile([1, 32], BF16)
        nc.vector.tensor_copy(ones_sb[:], ones_f[0:1, :].to_broadcast([1, 32]))
        # dummy custom-DVE op (output unused): routes DVE table generation
        # through the process-cached dve_table_for_ops path (~0.3s/call saved)
        dve_dummy = cpool.tile([1, 1], F32)
        nc.vector.reciprocal_approx_fast(dve_dummy[:], ones_f[0:1, 0:1])

        state = {}

        def p1_start(b):
            """input DMA (group-aligned slices) + fp8->bf16 casts + tile
            allocation for batch b. Cast g follows its own DMA slice; casts
            alternate DVE/GpSimd so no cast is queue-blocked by copies."""
            x8_sb = bpool.tile([C, S], F8, tag="x8")
            x_sb = bpool.tile([C, S], BF16, tag="x")
            pre_sb = bpool.tile([96, S], BF16, tag="pre")
            phm = wpool.tile([8, 2048], BF16, tag="phm")
            for gi, (g0, g1) in enumerate(GROUPS):
                nc.sync.dma_start(
                    x8_sb[:, g0 * SB:g1 * SB], xin[b][:, g0 * SB:g1 * SB])
                eng = nc.gpsimd if gi == 1 else nc.vector
                eng.tensor_copy(
                    x_sb[:, g0 * SB:g1 * SB], x8_sb[:, g0 * SB:g1 * SB])
            state[b] = {"x8": x8_sb, "x": x_sb, "pre": pre_sb, "phm": phm}

        def p1_conv(b, gi):
            """conv group gi for batch b: 3 matmuls -> copies.
            theta+phi rows copy on DVE (feeds pools/scores); g rows on GpSimd.
            The h-direction phi maxpool runs per group right after its copy."""
            st = state[b]
            x_sb, pre_sb, phm = st["x"], st["pre"], st["phm"]
            g0, g1 = GROUPS[gi]
            cps = ps_sc.tile([96, (g1 - g0) * SB], F32, tag="sc")
            for j in range(g0, g1):
                nc.tensor.matmul(
                    cps[:, (j - g0) * SB:(j - g0 + 1) * SB],
                    wcat_sb[:, 0:96], x_sb[:, j * SB:(j + 1) * SB],
                    start=True, stop=True,
                )
            nc.vector.tensor_copy(
                pre_sb[0:40, g0 * SB:g1 * SB], cps[0:40, :])
            # g rows drain on ACT (idle during startup; GpSimd can't read PSUM)
            nc.scalar.activation(
                pre_sb[64:96, g0 * SB:g1 * SB], cps[64:96, :],
                mybir.ActivationFunctionType.Copy)
            # phi h-max for this group's columns (cols are (h w) pairs in w)
            nrow = (g1 - g0) * SB // 64  # 64-wide w rows in this slice
            pv = pre_sb[32:40, g0 * SB:g1 * SB].rearrange(
                "p (h w) -> p h w", h=nrow)
            nc.vector.tensor_tensor(
                phm[:, g0 * SB // 2:g1 * SB // 2].rearrange(
                    "p (h w) -> p h w", h=nrow),
                pv[:, :, 0:64:2], pv[:, :, 1:64:2], AX.max)

        def p1_pools(b):
            """w-direction phi maxpool (DVE) + full g maxpool (GpSimd)."""
            st = state[b]
            pre_sb, phm = st["pre"], st["phm"]
            phi_sb = wpool.tile([8, T], BF16, tag="phi")
            g_sb = wpool.tile([32, T], BF16, tag="g")
            ghm = wpool.tile([32, 2048], BF16, tag="ghm")
            ph2 = phm[:].rearrange("p (h w) -> p h w", h=64)
            nc.vector.tensor_tensor(
                phi_sb[:].rearrange("p (h w) -> p h w", h=32),
                ph2[:, 0:64:2, :], ph2[:, 1:64:2, :], AX.max)
            gv = pre_sb[64:96].rearrange("p (h w) -> p h w", h=64)
            nc.vector.tensor_tensor(
                ghm[:].rearrange("p (h w) -> p h w", h=64),
                gv[:, :, 0:64:2], gv[:, :, 1:64:2], AX.max)
            gh2 = ghm[:].rearrange("p (h w) -> p h w", h=64)
            nc.vector.tensor_tensor(
                g_sb[:].rearrange("p (h w) -> p h w", h=32),
                gh2[:, 0:64:2, :], gh2[:, 1:64:2, :], AX.max)
            st["phi"] = phi_sb
            st["g"] = g_sb

        def phase1_g2t(b):
            """g2T chunks: [128 t, 33] = g[:, chunk].T via identity; col 32 =
            ones. Emitted after the first scores block of batch b so the PE
            queue starts scores as soon as phi is pooled."""
            g_sb = state[b]["g"]
            g2t_sb = bpool.tile([128, NTC * 33], BF16, tag="g2t")
            nc.gpsimd.tensor_copy(
                g2t_sb[:].rearrange("p (k c) -> p k c", c=33)[:, :, 32],
                ones_f[:].to_broadcast([128, NTC]))
            for k in range(NTC):
                g2ps = ps_o.tile([128, 32], F32, tag="o")
                nc.tensor.matmul(
                    g2ps[:], g_sb[:, k * 128:(k + 1) * 128], ident_sb[:],
                    start=True, stop=True,
                )
                nc.vector.tensor_copy(g2t_sb[:, k * 33:k * 33 + 32], g2ps[:])
            state[b]["g2t"] = g2t_sb

        def p2_scores(j, b):
            """scores -> exp for (j, b)."""
            pre_sb, phi_sb = state[b]["pre"], state[b]["phi"]
            theta = pre_sb[0:8]
            st_exp = epool.tile([128, NTC * SB], BF16, tag="stexp")
            for (k0, k1) in GROUPS:
                scps = ps_sc.tile([128, 3 * SB], F32, tag="sc")
                for k in range(k0, k1):
                    nc.tensor.matmul(
                        scps[:, (k - k0) * SB:(k - k0 + 1) * SB],
                        phi_sb[:, k * 128:(k + 1) * 128],
                        theta[:, j * SB:(j + 1) * SB],
                        start=True, stop=True,
                    )
                nc.scalar.activation(
                    st_exp[:, k0 * SB:k1 * SB],
                    scps[:, 0:(k1 - k0) * SB], EXP)
            return st_exp

        def p2_rest(j, b, st_exp):
            """o-matmul -> normalize -> DMA for (j, b)."""
            g2t_sb = state[b]["g2t"]
            o_ps = ps_o.tile([33, SB], F32, tag="o")
            for k in range(NTC):
                nc.tensor.matmul(
                    o_ps[:],
                    g2t_sb[:, k * 33:(k + 1) * 33],
                    st_exp[:, k * SB:(k + 1) * SB],
                    start=(k == 0), stop=(k == NTC - 1),
                )

            # normalize straight out of PSUM (no staging copy): the "o" ring
            # slot stays held until the mult reads it, which is still well
            # before the next-but-one o-matmul needs the bank. 1/Z fans out
            # across the 32 channel partitions on the GpSimd engine so the
            # mult has a single PSUM operand.
            zr = wpool.tile([1, SB], BF16, tag="zr")
            nc.vector.reciprocal(zr[:], o_ps[32:33, :])
            zb_sb = wpool.tile([32, SB], BF16, tag="zb")
            nc.gpsimd.partition_broadcast(zb_sb[:], zr[:])
            out_sb = wpool.tile([32, SB], F8, tag="out")
            nc.vector.tensor_tensor(out_sb[:], o_ps[0:32, :], zb_sb[:], AX.mult)
            nc.sync.dma_start(yout[b][:, j * SB:(j + 1) * SB], out_sb[:])

        # staggered schedule: batch 0's first scores start as early as
        # possible; g2t and batch 1's conv groups ride in the exp shadow of
        # batch 0's early j-blocks; then (j, b) pairs alternate so
        # PE/ACT/DVE/GpSimd stay fed
        p1_start(0)
        for gi in range(3):
            p1_conv(0, gi)
        p1_pools(0)
        se00 = p2_scores(0, 0)
        phase1_g2t(0)
        p1_start(1)
        p1_conv(1, 0)
        p2_rest(0, 0, se00)
        p1_conv(1, 1)
        se10 = p2_scores(1, 0)
        p1_conv(1, 2)
        p2_rest(1, 0, se10)
        p1_pools(1)
        se20 = p2_scores(2, 0)
        phase1_g2t(1)
        p2_rest(2, 0, se20)

        order = [(0, 1)]
        for j in range(3, NSB):
            order.append((j, 0))
            order.append((j - 2, 1))
        order.append((NSB - 2, 1))
        order.append((NSB - 1, 1))
        for (j, b) in order:
            se = p2_scores(j, b)
            p2_rest(j, b, se)


def kernel(x, w_theta, w_phi, w_g, w_o, gamma):
    global last_results
    x = np.ascontiguousarray(np.asarray(x, dtype=np.float32))
    B = x.shape[0]
    n_cores = 8
    per = B // n_cores
    assert per == NB

    if "prog" not in _prog_cache:
        _prog_cache["prog"] = _build_program()
    nc = _prog_cache["prog"]

    wcat_full = np.zeros((128, C), dtype=np.float32)
    wcat_full[0:8] = np.asarray(w_theta)
    wcat_full[32:40] = np.asarray(w_phi)
    wcat_full[64:96] = np.asarray(w_g)
    wcat_full[96:128, 0:32] = np.eye(32, dtype=np.float32)
    wcat_np = np.ascontiguousarray(wcat_full.T).astype(BF16_NP)

    x_f8 = x.reshape(B, C, S).astype(F8_NP)
    in_maps = [
        {
            "xin": x_f8[i * NB:(i + 1) * NB],
            "wcat": wcat_np,
        }
        for i in range(n_cores)
    ]
    res = run_bass_kernel_spmd(nc, in_maps, core_ids=list(range(n_cores)))
    last_results = res

    # o2 = (g @ beta^T)/Z from the device; w_o, gamma and the residual in f32
    o2f = np.empty((B, 32, S), dtype=np.float32)
    for i in range(n_cores):
        o2f[i * NB:(i + 1) * NB] = np.asarray(res.results[i]["yout"])
    w_og = (float(np.asarray(gamma)) * np.asarray(w_o)).astype(np.float32)
    out = np.matmul(w_og, o2f)
    out += x.reshape(B, C, S)
    return out.reshape(B, C, 64, 64)
